# revision 58
# baseline (speedup 1.0000x reference)
"""Trainium2 Bass kernel for DepthBranch: feat = relu(conv2(relu(conv1(x)))),
per-pixel argmin over depth hypotheses, one-hot scatter multiply into
(B, C, D, H, W) prior volume.

Sharding: 8 cores = (batch b in {0,1}) x (64-row H band q in {0..3}).
Each core computes its full band on-device and writes [C*D, 64*320].

v4 device algorithm per core:
  conv2's stationary operand is widened to 128 output rows so the PE
  directly emits feat4[r, p] = feat[r//4, p] (the feature map replicated
  4x across partitions, bf16).  Output row-tile t covers (c, dlo) =
  (r//4, r%4), depth d = 4t+dlo, DRAM row c*48+d.  Per-pixel argmin on
  ACT+DVE (exact f32, first-tie semantics match jnp.argmin); idx
  broadcast via PE into PSUM, cast to bf16 idxb on ACT.

  Output tiles ot = (idxb == dpat[t]) ? feat4 : 0 are produced by ONE
  custom DVE instruction (EQ_SEL, registered at import via the ant-dve
  table: IS_EQ + SELECT) with a hand-authored 2X_1PORT microcode
  program that processes a packed bf16 pair per lane-cycle (lo pair on
  ALU blocks 0-1, hi pair on blocks 2-3 via the SRC_*_HI input lanes,
  results carried on delay lanes 2/3 to the WR0_LO/WR0_HI write paths).
  This fuses the old is_equal + mult pair (4.4us) into 2.9us per tile.

  The output volume is written to DRAM as bf16 (the SBUF values are
  already bf16, so host-side f32 upcast in the gather is bit-identical
  to the old f32 cast-DMA at HALF the HBM write traffic).  Tile DMAs
  alternate between the SP HWDGE ring and Pool's SWDGE queue.

  Pipeline: group 0 is split in two sub-slabs (conv1 rho 0-8 + shift
  slab + conv2 rows 0-6 feed fills over pixels 0-2047 while the rest of
  the group is still in flight) to cut time-to-first-byte; groups 1-3
  run whole-group fills with the next group's conv staged at tile 2 and
  its idx broadcast at tile 6.
"""

import sys

for _p in ("/opt/trn_rl_repo", "/root/.axon_site/_ro/trn_rl_repo"):
    if _p not in sys.path:
        sys.path.insert(0, _p)

import numpy as np

import concourse.mybir as mybir
import concourse.tile as tile
from concourse.tile import add_dep_helper
from concourse import bacc
from concourse.bass_utils import run_bass_kernel_spmd

F32 = mybir.dt.float32
F32R = mybir.dt.float32r
BF16 = mybir.dt.bfloat16
U8 = mybir.dt.uint8
ALU = mybir.AluOpType
ACTF = mybir.ActivationFunctionType

# Problem geometry (hardcoded per spec nn_DepthBranch_42580305772560)
B, H, W, D, C, C1 = 2, 256, 320, 48, 32, 16
BAND = 64                     # H rows per core
PIX = BAND * W                # 20480 pixels per core
R = 16                        # rows per processing group
G = BAND // R                 # 4 groups
GPIX = R * W                  # 5120 pixels per group
FCOLS = PIX // 128            # 160 pixel-major columns
GF = GPIX // 128              # 40 f-columns per group
N = 1024                      # pi/idxb chunk (pixels)
NCH = GPIX // N               # 5 chunks per group
M = 2560                      # steady-state DVE op width (pixels)
NM = GPIX // M                # 2 per group
NT = (C * D) // 128           # 12 output row-tiles of 128
BIG = 64.0

_CACHE: dict = {}


def _ensure_eqsel():
    """Register a custom DVE op EQ_SEL: out = select(in1 == s0, in0, 0),
    fusing the fill's is_equal mask + mult into ONE instruction, with a
    hand-authored 2X_1PORT microcode program (packed-bf16 pair per cycle:
    lo on ALU blocks 0-1, hi on blocks 2-3 via the SRC_*_HI input lanes,
    results carried out on delay lanes 2/3 -> WR0_LO/WR0_HI)."""
    if "op" in _CACHE.setdefault("eqsel", {}):
        return _CACHE["eqsel"]["op"]
    import copy as _copy

    import concourse.dve_ops as dve_ops
    from concourse import dve_uop as du
    from concourse.dve_spec import C0, Spec, Src0, Src1, Zero, eq, lower, select

    spec = Spec(
        body=select(eq(Src1, C0), Src0, Zero),
        reference=lambda in0, in1, s0, s1, imm2: np.where(in1 == s0, in0, 0.0).astype(
            np.float32
        ),
    )
    uops1 = lower(spec, ver="v3")

    u2 = _copy.deepcopy(uops1[0])
    IS = du.InpSel
    u2.inp = [
        IS.ZERO, IS.SRC_1, IS.CONST_0, IS.SRC_0,
        IS.ZERO, IS.SRC_1_HI, IS.SRC_0_HI, IS.ZERO,
    ]
    u2.inp_enable = [0, 1, 1, 1, 1, 1, 1, 0]
    AI, DI, AO = du.AluInp, du.DelayInp, du.AluOp
    dp = [du.UopDpConfig() for _ in range(8)]

    def _blk(b, op=None, s0=None, s1=None, cap=None, fwd=()):
        blk = dp[b]
        if op is not None:
            blk.op, blk.alu_src0, blk.alu_src1 = op, s0, s1
        blk.alu_out_enable = 1
        for ln in fwd:
            blk.delay[ln] = DI.PREV_DELAY
            blk.delay_enable[ln] = 1
        if cap is not None:
            # latch the previous block's ALU result into this delay lane
            blk.delay[cap] = DI.PREV_ALU_OUT
            blk.delay_enable[cap] = 1

    # lanes: chain0=SRC_1(lo) chain1=C0 chain2=SRC_0(lo) chain3=ZERO
    #        chain4=SRC_1_HI chain5=SRC_0_HI
    _blk(0, AO.IS_EQ, AI.PREV_DELAY_0, AI.PREV_DELAY_1, fwd=(1, 2, 3, 4, 5))
    _blk(1, AO.SELECT, AI.PREV_DELAY_3, AI.PREV_DELAY_2, fwd=(1, 3, 4, 5))
    _blk(2, AO.IS_EQ, AI.PREV_DELAY_4, AI.PREV_DELAY_1, cap=2, fwd=(3, 5))
    _blk(3, AO.SELECT, AI.PREV_DELAY_3, AI.PREV_DELAY_5, fwd=(2,))
    _blk(4, cap=3, fwd=(2,))
    _blk(5, fwd=(2, 3))
    _blk(6, fwd=(2, 3))
    _blk(7, fwd=(2, 3))
    u2.datapath_config = dp
    u2.out = {p: du.OutSel.ALU_OUT for p in du.OutPath}
    u2.out_enable = {p: 0 for p in du.OutPath}
    u2.out[du.OutPath.WR0_LO] = du.OutSel.DELAY_2
    u2.out_enable[du.OutPath.WR0_LO] = 1
    u2.out[du.OutPath.WR0_HI] = du.OutSel.DELAY_3
    u2.out_enable[du.OutPath.WR0_HI] = 1

    name = "EQ_SEL_ANT_V1"
    row = dve_ops._CUSTOM_DVE_ROW_BASE + len(dve_ops.OPS)
    sspec = du.DveOpSpec(
        name=name,
        uops=uops1,
        opcode=row,
        uops_2x=[u2],
        perf_max=1,
        rd1_en=True,
    )

    class _Op:
        pass

    op = _Op()
    op.name = name
    op.spec = spec
    op.subdim = False
    op.uops_sha = {}
    op.perf_en = {"v3": True}
    op.compile = lambda ver, _s=sspec: _s
    dve_ops.OPS.append(op)
    dve_ops.CUSTOM_DVE_SPECS[name] = spec
    dve_ops._SUB_OPCODE_FOR_NAME[name] = row
    _CACHE["eqsel"]["op"] = op
    return op


def _build_nc(reps=1):
    eqsel_op = _ensure_eqsel()
    nc = bacc.Bacc(None, target_bir_lowering=False)

    x9_d = nc.dram_tensor("x9", [9, 66 * 322], BF16, kind="ExternalInput")
    xpm_d = nc.dram_tensor("xpm", [128, FCOLS], F32, kind="ExternalInput")
    hypb_d = nc.dram_tensor("hypB", [128, D], F32, kind="ExternalInput")
    w1t_d = nc.dram_tensor("w1T", [9, C1], BF16, kind="ExternalInput")
    b1m_d = nc.dram_tensor("b1m", [C1, G * (R + 2)], F32, kind="ExternalInput")
    rmsk_d = nc.dram_tensor("rmask", [C1, G * (R + 2)], F32, kind="ExternalInput")
    w2t4_d = nc.dram_tensor("w2T4", [3 * C1, 3 * 128], BF16, kind="ExternalInput")
    b2c4_d = nc.dram_tensor("b2c4", [128, 1], F32, kind="ExternalInput")
    dpat4_d = nc.dram_tensor("dpat4", [128, NT], F32, kind="ExternalInput")
    iotb_d = nc.dram_tensor("iotaBIG", [128, D], F32, kind="ExternalInput")
    ident_d = nc.dram_tensor("ident", [128, 128], BF16, kind="ExternalInput")
    # out rows ordered (c, t, dlo) -> DRAM row c*48 + 4t + dlo
    # bf16: SBUF values are already bf16, so writing bf16 + host-side
    # f32 cast is bit-identical to the old f32 cast-DMA at half the HBM
    # write traffic (the roofline resource).
    out_d = nc.dram_tensor("out", [C, NT, 4, PIX], BF16, kind="ExternalOutput")

    with tile.TileContext(nc) as tc:
        with (
            tc.tile_pool(name="const", bufs=1) as constp,
            tc.tile_pool(name="x9p", bufs=2) as x9p,
            tc.tile_pool(name="featp", bufs=2) as featp,
            tc.tile_pool(name="argm", bufs=3) as argm,
            tc.tile_pool(name="drawp", bufs=4) as drawp,
            tc.tile_pool(name="argv", bufs=2) as argv,
            tc.tile_pool(name="idxp", bufs=4) as idxp,
            tc.tile_pool(name="idxbp", bufs=2) as idxbp,
            tc.tile_pool(name="otp", bufs=4) as otp,
            tc.tile_pool(name="psI", bufs=2, space="PSUM") as psI,
            tc.tile_pool(name="psC", bufs=4, space="PSUM") as psC,
        ):
            # --- load constants (order = criticality) ---
            def ld(dram, shape, tag, dt=F32):
                t = constp.tile(shape, dt, tag=tag)
                nc.sync.dma_start(out=t[:], in_=dram[:])
                return t

            xpm = ld(xpm_d, [128, FCOLS], "xpm")
            hypb = ld(hypb_d, [128, D], "hypb")
            iotb = ld(iotb_d, [128, D], "iotb")
            ident = ld(ident_d, [128, 128], "ident", BF16)
            # group 0's x9 slab ahead of the remaining constants so conv1
            # can start immediately
            x9_first = x9p.tile([9, (R + 2) * 322], BF16, tag="x9", name="x9_0")
            nc.sync.dma_start(out=x9_first[:], in_=x9_d[:, 0 : (R + 2) * 322])
            w1t = ld(w1t_d, [9, C1], "w1t", BF16)
            b1m = ld(b1m_d, [C1, G * (R + 2)], "b1m")
            rmsk = ld(rmsk_d, [C1, G * (R + 2)], "rmsk")
            w2t4 = ld(w2t4_d, [3 * C1, 3 * 128], "w2t4", BF16)
            b2c4 = ld(b2c4_d, [128, 1], "b2c4")
            dpat4 = ld(dpat4_d, [128, NT], "dpat4")

            # PE warmup: ramp the HAM clock gate while input DMAs land, so
            # conv1/conv2 of group 0 run at full clock.
            for wi in range(3):
                wps = psI.tile([128, N], F32, tag="pi", name=f"warm_{wi}")
                nc.tensor.matmul(
                    wps[:, 0:128],
                    ident[:],
                    ident[:],
                    start=True,
                    stop=True,
                )

            # x2 double buffer (persistent tiles): conv1 output (bf16) with
            # 3 dx-shifted copies stacked on partition blocks, stored
            # 320-wide so conv2 row-pairs are contiguous moving operands.
            # Partition block 0 = plain x2[j] (conv1 writes here), block 1
            # = shift-right x2[j-1] (col 0 = zero halo), block 2 =
            # shift-left x2[j+1] (col 319 = zero halo).  w2T4 rows follow
            # this (dx=1, dx=0, dx=2) order.  Halo columns are zeroed once
            # here and never rewritten.
            x2bufs = []
            for xb in range(2):
                x2b = constp.tile([3 * C1, R + 2, W], BF16, tag=f"x2_{xb}")
                # partition-start must be 32-aligned: cover blocks 0+1 for
                # col 0 (block 0's col 0 is overwritten by conv1 anyway)
                nc.vector.memset(x2b[0 : 2 * C1, :, 0:1], 0.0)
                nc.vector.memset(x2b[2 * C1 : 3 * C1, :, W - 1 : W], 0.0)
                x2bufs.append(x2b)

            feats = {}
            idxs = {}
            x9held = {}
            conv2_mms = {}

            def load_x9(g):
                r0 = R * g
                x9_g = x9p.tile([9, (R + 2) * 322], BF16, tag="x9", name=f"x9_{_rep}_{g}")
                nc.sync.dma_start(
                    out=x9_g[:], in_=x9_d[:, r0 * 322 : (r0 + R + 2) * 322]
                )
                return x9_g

            def emit_conv(g, x9_g=None, part=None):
                # conv1: rows r0-1 .. r0+R (18 conv1-grid rows).  Halo rows
                # (rho 0, 17) may be out-of-image and get per-row validity
                # masks.  part="A"/"B" splits the group in two so group 0's
                # first feat rows (and hence the output stream) are ready
                # ~15us earlier; part=None emits the whole group.
                if part == "A":
                    rhos, slabs, rows = range(0, 9), ((0, 9),), range(0, 7)
                elif part == "B":
                    rhos, slabs, rows = range(9, R + 2), ((9, R + 2),), range(7, R)
                else:
                    rhos, slabs, rows = range(R + 2), ((0, 6), (6, 12), (12, R + 2)), range(R)
                if x9_g is None:
                    x9_g = x9held.get(g)
                if x9_g is None:
                    x9_g = load_x9(g)
                x9held[g] = x9_g
                x2_3 = x2bufs[g % 2]

                for rho in rhos:
                    p1 = psC.tile([C1, 322], F32, tag="c", name=f"p1_{_rep}_{g}_{rho}")
                    nc.tensor.matmul(
                        p1[:],
                        w1t[:],
                        x9_g[:, rho * 322 : (rho + 1) * 322],
                        start=True,
                        stop=True,
                    )
                    col = g * (R + 2) + rho
                    nc.scalar.activation(
                        x2_3[0:C1, rho, 0:W],
                        p1[:, 1:321],
                        ACTF.Relu,
                        scale=rmsk[:, col : col + 1],
                        bias=b1m[:, col : col + 1],
                    )
                # dx-shifted partition copies for K=48 conv2 taps, in row
                # slabs so conv2 rows can start before conv1 fully finishes.
                # On SP HWDGE so they never touch the output-DMA stream's
                # SDMA time materially.
                for r0s, r1s in slabs:
                    nc.sync.dma_start(
                        out=x2_3[C1 : 2 * C1, r0s:r1s, 1:W],
                        in_=x2_3[0:C1, r0s:r1s, 0 : W - 1],
                    )
                    nc.sync.dma_start(
                        out=x2_3[2 * C1 : 3 * C1, r0s:r1s, 0 : W - 1],
                        in_=x2_3[0:C1, r0s:r1s, 1:W],
                    )
                # conv2: R rows of feat4 (feat pre-replicated 4x across
                # partitions: row r holds channel r//4), 3 accumulating K=48
                # matmuls per row.
                if g in feats:
                    feat_g = feats[g]
                else:
                    feat_g = featp.tile([128, GPIX], BF16, tag="feat", name=f"feat_{_rep}_{g}")
                    feats[g] = feat_g
                x2f = x2_3[:].rearrange("p r x -> p (r x)")
                for r in rows:
                    p2 = psC.tile([128, W], F32, tag="c", name=f"p2_{_rep}_{g}_{r}")
                    for dy in range(3):
                        nc.tensor.matmul(
                            p2[:],
                            w2t4[:, dy * 128 : (dy + 1) * 128],
                            x2f[:, (r + dy) * W : (r + dy + 1) * W],
                            start=(dy == 0),
                            stop=(dy == 2),
                        )
                    nc.scalar.activation(
                        feat_g[:, r * W : (r + 1) * W],
                        p2[:],
                        ACTF.Relu,
                        bias=b2c4[:],
                    )

            draws = {}

            def emit_draw(g):
                # diff[p,f,d] = hyp[d] - x[p,f] via dual broadcast APs on
                # GPSIMD.  Emitted well before the rest of the argmin chain
                # so Q7 drains never interrupt the output-DMA stream.
                HF = GF // 2
                ds = []
                for h in range(2):
                    f0 = g * GF + h * HF
                    draw = drawp.tile(
                        [128, HF, D], F32, tag="dr", name=f"draw_{_rep}_{g}_{h}"
                    )
                    nc.gpsimd.tensor_tensor(
                        out=draw[:],
                        in0=hypb[:]
                        .rearrange("p (o d) -> p o d", o=1)
                        .broadcast_to((128, HF, D)),
                        in1=xpm[:, f0 : f0 + HF]
                        .rearrange("p (f o) -> p f o", o=1)
                        .broadcast_to((128, HF, D)),
                        op=ALU.subtract,
                    )
                    ds.append(draw)
                draws[g] = ds

            def emit_argmin(g, after=None, halves=(0, 1)):
                # per-pixel argmin over D hypotheses (pixel-major, exact f32),
                # split into two half-group chains to cut critical latency.
                HF = GF // 2
                idx_tiles = idxs.get(g, [None, None])
                for h in halves:
                    draw = draws[g][h]
                    diff = argm.tile(
                        [128, HF, D], F32, tag="a3", name=f"diff_{_rep}_{g}_{h}"
                    )
                    abs_i = nc.scalar.activation(diff[:], draw[:], ACTF.Abs)
                    minv = argv.tile([128, HF], F32, tag="av", name=f"minv_{_rep}_{g}_{h}")
                    nc.vector.tensor_reduce(
                        out=minv[:], in_=diff[:], axis=mybir.AxisListType.X,
                        op=ALU.min,
                    )
                    eq = argm.tile([128, HF, D], F32, tag="a3", name=f"eq_{_rep}_{g}_{h}")
                    nc.vector.tensor_tensor(
                        out=eq[:],
                        in0=diff[:],
                        in1=minv[:]
                        .rearrange("p (f o) -> p f o", o=1)
                        .broadcast_to((128, HF, D)),
                        op=ALU.is_equal,
                    )
                    cand = argm.tile(
                        [128, HF, D], F32, tag="a3", name=f"cand_{_rep}_{g}_{h}"
                    )
                    nc.vector.scalar_tensor_tensor(
                        out=cand[:],
                        in0=eq[:],
                        scalar=-BIG,
                        in1=iotb[:]
                        .rearrange("p (o d) -> p o d", o=1)
                        .broadcast_to((128, HF, D)),
                        op0=ALU.mult,
                        op1=ALU.add,
                    )
                    # idx in bf16 (small integers -> exact) so the broadcast
                    # matmul in the fill runs at full-rate bf16.
                    idx_h = idxp.tile([128, HF], BF16, tag="avr", name=f"idx_{_rep}_{g}_{h}")
                    nc.vector.tensor_reduce(
                        out=idx_h[:], in_=cand[:], axis=mybir.AxisListType.X,
                        op=ALU.min,
                    )
                    idx_tiles[h] = idx_h
                idxs[g] = idx_tiles

            def emit_pi_chunk(g, idxb, cch):
                pi = psI.tile([128, N], F32, tag="pi", name=f"pi_{_rep}_{g}_{cch}")
                for j in range(N // 128):
                    fc = cch * (N // 128) + j
                    idx_h = idxs[g][fc // (GF // 2)]
                    fl = fc % (GF // 2)
                    # pi[:, 128j+p] = idx[p, fc] for all rows
                    nc.tensor.matmul(
                        pi[:, j * 128 : (j + 1) * 128],
                        idx_h[:, fl : fl + 1].broadcast_to((128, 128)),
                        ident[:],
                        start=True,
                        stop=True,
                    )
                nc.scalar.copy(out=idxb[:, cch * N : (cch + 1) * N], in_=pi[:])

            def emit_fill(g, idxb, t, lo, hi):
                # fused fill: ot = (idxb == dpat[t]) ? feat : 0 in ONE
                # custom-DVE op running the hand-authored 2X_1PORT program;
                # bf16->bf16 DMA split across the SP HWDGE ring and Pool's
                # SWDGE queue.
                w = hi - lo
                feat_g = feats[g]
                ot = otp.tile([128, w], BF16, tag="ot", name=f"ot_{_rep}_{g}_{t}_{lo}")
                ts = nc.vector._custom_dve(
                    eqsel_op,
                    out=ot[:],
                    in0=feat_g[:, lo:hi],
                    in1=idxb[:, lo:hi],
                    s0=dpat4[:, t : t + 1],
                )
                try:
                    ts.ins.perf_max = 1
                except Exception:
                    pass
                dma_eng = nc.sync if (t % 2 == 0) else nc.gpsimd
                dma_eng.dma_start(
                    out=out_d[:, t, :, g * GPIX + lo : g * GPIX + hi],
                    in_=ot[:],
                )
                return ts

            def stage_idxb(g, chunks=range(NCH)):
                if g in idxbs:
                    idxb = idxbs[g]
                else:
                    idxb = idxbp.tile([128, GPIX], BF16, tag="ib", name=f"ib_{_rep}_{g}")
                    idxbs[g] = idxb
                for cch in chunks:
                    emit_pi_chunk(g, idxb, cch)
                return idxb

            for _rep in range(reps):
                idxbs = {}
                # --- group 0 prologue: split into two sub-slabs so the
                # output stream starts as soon as conv2 rows 0-7 and the
                # half-0 argmin are done (~15us earlier than a full group).
                emit_draw(0)
                emit_argmin(0, halves=(0,))
                emit_conv(0, x9_g=x9_first if _rep == 0 else None, part="A")
                emit_argmin(0, halves=(1,))
                idxb0 = stage_idxb(0, chunks=(0, 1))
                emit_conv(0, part="B")
                stage_idxb(0, chunks=(2, 3, 4))
                emit_draw(1)
                x9_next = load_x9(1)
                # sub-A fills: pixels [0, 2048) need only conv2 rows 0-6
                # and idxb chunks 0-1.
                SPL = 2 * N
                first_ts = None
                for t in range(NT):
                    ts = emit_fill(0, idxb0, t, 0, SPL)
                    if first_ts is None:
                        first_ts = ts
                    if t == 1:
                        emit_argmin(1, after=first_ts)
                # sub-B fills: rest of group 0
                for t in range(NT):
                    if t == 2:
                        emit_conv(1, x9_g=x9_next)
                    if t == 6:
                        stage_idxb(1)
                    emit_fill(0, idxb0, t, SPL, GPIX)

                for g in range(1, G):
                    idxb = idxbs[g] if g in idxbs else stage_idxb(g)
                    if g + 1 < G:
                        emit_draw(g + 1)
                        x9_next = load_x9(g + 1)
                    first_ts = None
                    for t in range(NT):
                        if t == 1 and g + 1 < G:
                            emit_argmin(g + 1, after=first_ts)
                        if t == 2 and g + 1 < G:
                            emit_conv(g + 1, x9_g=x9_next)
                        if t == 5 and g + 1 < G:
                            # stage next group's idx broadcast now so its ACT
                            # copies land ahead of the next conv relu batch
                            # and fills(g+1) never wait at the boundary
                            stage_idxb(g + 1)
                        ts = emit_fill(g, idxb, t, 0, GPIX)
                        if first_ts is None:
                            first_ts = ts
    nc.compile()
    return nc


def _consts(w1, b1, w2, b2):
    import ml_dtypes
    w1T = np.ascontiguousarray(w1.reshape(C1, 9).T.astype(ml_dtypes.bfloat16))
    # w2T3[dx*16+cin, dy*32+co] = w2[co, cin, dy, dx]
    w2T3 = np.ascontiguousarray(
        w2.transpose(3, 1, 2, 0).reshape(3 * C1, 3 * C), dtype=np.float32
    )
    # widen to 128 output rows: row r of the conv2 output holds channel r//4
    w2T4 = np.zeros((3 * C1, 3 * 128), np.float32)
    perm = np.r_[16:32, 0:16, 32:48]  # partition blocks (dx=1, dx=0, dx=2)
    for dy in range(3):
        blk = w2T3[perm][:, dy * C : (dy + 1) * C]  # [48, 32]
        w2T4[:, dy * 128 : (dy + 1) * 128] = np.repeat(blk, 4, axis=1)
    b2c4 = np.ascontiguousarray(np.repeat(b2, 4).reshape(128, 1), dtype=np.float32)
    import ml_dtypes
    w2T4 = w2T4.astype(ml_dtypes.bfloat16)
    rr = np.arange(128)
    dpat4 = np.stack([4 * t + (rr % 4) for t in range(NT)], axis=1).astype(np.float32)
    iotb = np.tile((np.arange(D) + BIG).astype(np.float32)[None, :], (128, 1))
    ident = np.eye(128, dtype=np.float32).astype(ml_dtypes.bfloat16)
    return dict(
        w2T4=np.ascontiguousarray(w2T4), b2c4=b2c4,
        dpat4=np.ascontiguousarray(dpat4),
        iotaBIG=np.ascontiguousarray(iotb),
        ident=ident, w1T=w1T,
    )


def _in_maps(ref_init_depth, depth_hypotheses, w1, b1, w2, b2):
    consts = _consts(
        np.asarray(w1, np.float32), np.asarray(b1, np.float32),
        np.asarray(w2, np.float32), np.asarray(b2, np.float32),
    )
    x = np.asarray(ref_init_depth, np.float32)
    hyp = np.asarray(depth_hypotheses, np.float32)
    b1f = np.asarray(b1, np.float32)

    in_maps = []
    for k in range(8):
        b, q = k // 4, k % 4
        h0 = BAND * q
        xb = x[b, 0]  # (H, W)
        xp = np.zeros((BAND + 4, W + 4), np.float32)
        lo, hi = max(0, h0 - 2), min(H, h0 + BAND + 2)
        xp[lo - (h0 - 2) : hi - (h0 - 2), 2 : 2 + W] = xb[lo:hi]
        import ml_dtypes
        x9 = np.stack(
            [xp[dy : dy + BAND + 2, dx : dx + W + 2] for dy in range(3) for dx in range(3)]
        ).reshape(9, (BAND + 2) * (W + 2)).astype(ml_dtypes.bfloat16)
        band = xb[h0 : h0 + BAND].reshape(PIX)
        xpm = np.ascontiguousarray(band.reshape(FCOLS, 128).T)
        hypB = np.tile(hyp[b][None, :], (128, 1))
        # conv1-row validity mask: image row = h0 + R*g - 1 + rho
        m = np.zeros(G * (R + 2), np.float32)
        for g in range(G):
            for rho in range(R + 2):
                img = h0 + R * g - 1 + rho
                m[g * (R + 2) + rho] = 1.0 if 0 <= img < H else 0.0
        rmask = np.tile(m[None, :], (C1, 1))
        b1m = b1f.reshape(C1, 1) * rmask
        in_maps.append(
            dict(
                x9=np.ascontiguousarray(x9),
                xpm=xpm,
                hypB=np.ascontiguousarray(hypB),
                b1m=np.ascontiguousarray(b1m),
                rmask=np.ascontiguousarray(rmask),
                **consts,
            )
        )
    return in_maps


def kernel(ref_init_depth, depth_hypotheses, w1, b1, w2, b2):
    if "nc" not in _CACHE:
        _CACHE["nc"] = _build_nc()
    nc = _CACHE["nc"]

    in_maps = _in_maps(ref_init_depth, depth_hypotheses, w1, b1, w2, b2)

    import os
    trace = os.environ.get("BASS_TRACE", "0") == "1"
    trace_cores = None
    if os.environ.get("BASS_TRACE_ALL", "0") == "1":
        trace_cores = list(range(8))
    res = run_bass_kernel_spmd(
        nc, in_maps, core_ids=list(range(8)), trace=trace, trace_cores=trace_cores
    )
    _CACHE["last_results"] = res
    out = np.empty((B, C, D, H, W), np.float32)
    for k in range(8):
        b, q = k // 4, k % 4
        out[b, :, :, BAND * q : BAND * (q + 1), :] = (
            res.results[k]["out"].reshape(C, D, BAND, W).astype(np.float32)
        )
    return out

